# revision 1
# baseline (speedup 1.0000x reference)
"""DEMONetHashGraph Trainium2 kernel — 8-core data-parallel GNN.

Strategy:
- Fold the multi-hash einsum+concat+Wp into a single [512,512] weight on host:
  hashed @ Wp == agg @ (hstack(Hm) @ Wp).
- Shard nodes (and their outgoing edges) across 8 cores by contiguous range.
- Per layer: segment-mean via one-hot matmuls in PSUM over src-sorted edge
  tiles (gathered neighbor rows via indirect DMA, bf16), then two fp32r dense
  matmuls (hash path on agg, self path on h) + bias + ELU.
- One bf16 AllGather of h1 between layers; tiny AllReduce for per-graph pools.
"""

import sys

for _p in ("/opt/trn_rl_repo", "/root/.axon_site/_ro/trn_rl_repo"):
    if _p not in sys.path:
        sys.path.insert(0, _p)

import time
from contextlib import ExitStack

import ml_dtypes
import numpy as np

import concourse.bass as bass
import concourse.mybir as mybir
import concourse.tile as tile
from concourse import bacc
from concourse.masks import make_identity

# problem constants (hardcoded per spec)
N_NODES = 50000
N_EDGES = 800000
D = 512
NUM_GRAPHS = 64
NUM_CLASSES = 10
NC = 8
SHARD = N_NODES // NC  # 6250
BN = 128
NB = (SHARD + BN - 1) // BN  # 49
SHARD_PAD = NB * BN  # 6272
NPAD = NC * SHARD_PAD  # 50176
SPLIT = 32768

f32 = mybir.dt.float32
f32r = mybir.dt.float32r
bf16 = mybir.dt.bfloat16
i32 = mybir.dt.int32
BF = ml_dtypes.bfloat16


def _preprocess(x, edge_index, batch, Hm1, Wp1, Ws1, b1, Hm2, Wp2, Ws2, b2, Wc, bc):
    x = np.asarray(x, np.float32)
    src = np.asarray(edge_index[0], np.int64)
    dst = np.asarray(edge_index[1], np.int64)
    batch = np.asarray(batch, np.int64)

    deg = np.bincount(src, minlength=N_NODES)
    iso = np.where(deg == 0)[0]
    if iso.size:
        src = np.concatenate([src, iso])
        dst = np.concatenate([dst, iso])

    order = np.argsort(src, kind="stable")
    src_s = src[order]
    dst_s = dst[order]
    # remap dst to padded (per-core) row coordinates
    dst_pad = (dst_s // SHARD) * SHARD_PAD + (dst_s % SHARD)

    # per-(core, block) edge ranges
    blk_starts = []
    for c in range(NC):
        for b in range(NB):
            blk_starts.append(c * SHARD + b * BN)
    blk_starts.append(N_NODES)
    bounds = np.searchsorted(src_s, np.array(blk_starts))

    # split each block's edges by gather-table half (int16 index limit)
    lo_mask = dst_pad < SPLIT
    cnt_lo = np.zeros((NC, NB), np.int64)
    cnt_hi = np.zeros((NC, NB), np.int64)
    for c in range(NC):
        for b in range(NB):
            k = c * NB + b
            m = lo_mask[bounds[k] : bounds[k + 1]]
            cnt_lo[c, b] = int(m.sum())
            cnt_hi[c, b] = int((~m).sum())
    T_lo = max(1, int(np.max((cnt_lo + BN - 1) // BN)))
    T_hi = max(1, int(np.max((cnt_hi + BN - 1) // BN)))
    T = T_lo + T_hi

    slot_arr = np.full((NC, BN, NB * T), -1.0, np.float32)
    ilo = np.zeros((NC, 16, NB * T_lo * 8), np.int16)
    ihi = np.zeros((NC, 16, NB * T_hi * 8), np.int16)
    for c in range(NC):
        for b in range(NB):
            k = c * NB + b
            e0, e1 = bounds[k], bounds[k + 1]
            m = lo_mask[e0:e1]
            d_blk = dst_pad[e0:e1]
            s_blk = (src_s[e0:e1] - (c * SHARD + b * BN)).astype(np.float32)
            for half, sel, base_t, tt in (
                (0, m, 0, T_lo),
                (1, ~m, T_lo, T_hi),
            ):
                d = d_blk[sel] - (SPLIT if half else 0)
                s = s_blk[sel]
                n = len(d)
                if n:
                    j = np.arange(n)
                    slot_arr[c, j % BN, b * T + base_t + j // BN] = s
                    arr = ilo if half == 0 else ihi
                    arr[c, j % 16, b * tt * 8 + j // 16] = d.astype(np.int16)

    # inv deg / batch slots per (core, partition, block)
    node_idx = (
        np.arange(NC)[:, None, None] * SHARD
        + np.arange(NB)[None, None, :] * BN
        + np.arange(BN)[None, :, None]
    )  # [NC, BN, NB]
    valid = node_idx < (np.arange(NC)[:, None, None] + 1) * SHARD
    node_clip = np.minimum(node_idx, N_NODES - 1)
    invdeg = np.where(valid, 1.0 / np.maximum(deg[node_clip], 1), 1.0).astype(
        np.float32
    )
    bslot = np.where(valid, batch[node_clip].astype(np.float32), -1.0).astype(
        np.float32
    )

    cnt_g = np.bincount(batch, minlength=NUM_GRAPHS).astype(np.float32)
    invcnt = np.broadcast_to(
        (1.0 / np.maximum(cnt_g, 1.0))[None, :], (BN, NUM_GRAPHS)
    ).copy()

    # padded x (gather table, bf16) and per-core fp32 shards
    x_pad = np.zeros((NPAD, D), np.float32)
    x_pad_view = x_pad.reshape(NC, SHARD_PAD, D)
    x_pad_view[:, :SHARD, :] = x.reshape(NC, SHARD, D)
    x_bf = x_pad.astype(BF)
    x_shards = [np.ascontiguousarray(x_pad_view[c]) for c in range(NC)]

    # folded weights
    def fold(Hm, Wp):
        Hcat = np.concatenate([np.asarray(Hm, np.float32)[k] for k in range(4)], axis=1)
        return (Hcat @ np.asarray(Wp, np.float32)).astype(np.float32)

    w = dict(
        w1a=fold(Hm1, Wp1),
        wsa=np.asarray(Ws1, np.float32),
        w1b=fold(Hm2, Wp2),
        wsb=np.asarray(Ws2, np.float32).astype(BF),
        wc=np.asarray(Wc, np.float32),
        b1=np.asarray(b1, np.float32).reshape(1, D),
        b2=np.asarray(b2, np.float32).reshape(1, D),
        bc=np.asarray(bc, np.float32).reshape(1, NUM_CLASSES),
        ones=np.ones((1, BN), np.float32),
    )
    return dict(
        T=T,
        T_lo=T_lo,
        T_hi=T_hi,
        ilo=ilo,
        ihi=ihi,
        slot=slot_arr.astype(BF),
        invdeg=invdeg,
        bslot=bslot,
        invcnt=invcnt,
        x_bf=x_bf,
        x_shards=x_shards,
        w=w,
    )


def _build(T, T_lo, T_hi, reps=1, ablate=(), qmode=0):
    nc = bacc.Bacc(
        "TRN2",
        target_bir_lowering=False,
        debug=False,
        num_devices=NC,
        num_swdge_queues=2,
    )

    ein = dict(kind="ExternalInput")
    xg_d = nc.dram_tensor("xg", [NPAD, D], bf16, **ein)
    xs_d = nc.dram_tensor("xs", [SHARD_PAD, D], f32, **ein)
    ilo_d = nc.dram_tensor("ilo", [16, NB * T_lo * 8], mybir.dt.int16, **ein)
    ihi_d = nc.dram_tensor("ihi", [16, NB * T_hi * 8], mybir.dt.int16, **ein)
    slot_d = nc.dram_tensor("slot", [BN, NB * T], bf16, **ein)
    invdeg_d = nc.dram_tensor("invdeg", [BN, NB], f32, **ein)
    bslot_d = nc.dram_tensor("bslot", [BN, NB], f32, **ein)
    invcnt_d = nc.dram_tensor("invcnt", [BN, NUM_GRAPHS], f32, **ein)
    w1a_d = nc.dram_tensor("w1a", [D, D], f32r, **ein)
    wsa_d = nc.dram_tensor("wsa", [D, D], f32r, **ein)
    w1b_d = nc.dram_tensor("w1b", [D, D], f32r, **ein)
    wsb_d = nc.dram_tensor("wsb", [D, D], bf16, **ein)
    wc_d = nc.dram_tensor("wc", [D, NUM_CLASSES], f32r, **ein)
    b1_d = nc.dram_tensor("b1", [1, D], f32r, **ein)
    b2_d = nc.dram_tensor("b2", [1, D], f32r, **ein)
    bc_d = nc.dram_tensor("bc", [1, NUM_CLASSES], f32r, **ein)
    ones_d = nc.dram_tensor("ones", [1, BN], f32r, **ein)
    out_d = nc.dram_tensor("out", [NUM_GRAPHS, NUM_CLASSES], f32, kind="ExternalOutput")

    with tile.TileContext(nc) as tc, ExitStack() as ctx:
        const = ctx.enter_context(tc.tile_pool(name="const", bufs=1))
        dram = ctx.enter_context(tc.tile_pool(name="dram", bufs=1, space="DRAM"))
        gpool = ctx.enter_context(tc.tile_pool(name="gpool", bufs=3))
        spool = ctx.enter_context(tc.tile_pool(name="spool", bufs=2))
        work = ctx.enter_context(tc.tile_pool(name="work", bufs=2))
        hpool = ctx.enter_context(tc.tile_pool(name="hpool", bufs=3))
        ps_nsum = ctx.enter_context(tc.tile_pool(name="ps_nsum", bufs=2, space="PSUM"))
        ps_tr = ctx.enter_context(tc.tile_pool(name="ps_tr", bufs=2, space="PSUM"))
        ps_dense = ctx.enter_context(
            tc.tile_pool(name="ps_dense", bufs=2, space="PSUM")
        )
        ps_pool = ctx.enter_context(tc.tile_pool(name="ps_pool", bufs=1, space="PSUM"))
        ps_fin = ctx.enter_context(tc.tile_pool(name="ps_fin", bufs=1, space="PSUM"))

        # ---- constants / loads ----
        ident = const.tile([BN, BN], f32)
        make_identity(nc, ident[:])
        iota_i = const.tile([BN, BN], i32)
        nc.gpsimd.iota(iota_i[:], pattern=[[1, BN]], base=0, channel_multiplier=0)
        iota_bf = const.tile([BN, BN], bf16)
        nc.vector.tensor_copy(iota_bf[:], iota_i[:])
        iota_g = const.tile([BN, NUM_GRAPHS], f32)
        nc.vector.tensor_copy(iota_g[:], iota_i[:, :NUM_GRAPHS])

        ilo_sb = const.tile([BN, NB * T_lo * 8], mybir.dt.int16)
        ihi_sb = const.tile([BN, NB * T_hi * 8], mybir.dt.int16)
        for rep8 in range(8):
            nc.sync.dma_start(ilo_sb[rep8 * 16 : (rep8 + 1) * 16, :], ilo_d[:, :])
            nc.sync.dma_start(ihi_sb[rep8 * 16 : (rep8 + 1) * 16, :], ihi_d[:, :])
        slot_sb = const.tile([BN, NB * T], bf16)
        nc.sync.dma_start(slot_sb[:], slot_d[:, :])
        invdeg_sb = const.tile([BN, NB], f32)
        nc.sync.dma_start(invdeg_sb[:], invdeg_d[:, :])
        bslot_sb = const.tile([BN, NB], f32)
        nc.sync.dma_start(bslot_sb[:], bslot_d[:, :])
        invcnt_sb = const.tile([BN, NUM_GRAPHS], f32)
        nc.sync.dma_start(invcnt_sb[:], invcnt_d[:, :])

        def load_w(dram_t):
            t = const.tile([BN, 4, D], f32r, name=f"w_{dram_t.name}")
            nc.sync.dma_start(t[:], dram_t[:, :].rearrange("(ks kp) n -> kp ks n", kp=BN))
            return t

        w1a_sb = load_w(w1a_d)
        wsa_sb = load_w(wsa_d)
        w1b_sb = load_w(w1b_d)
        wsb_sb = const.tile([BN, 4, D], bf16)
        nc.sync.dma_start(
            wsb_sb[:], wsb_d[:, :].rearrange("(ks kp) n -> kp ks n", kp=BN)
        )
        wc_sb = const.tile([BN, 4, NUM_CLASSES], f32r)
        nc.sync.dma_start(wc_sb[:], wc_d[:, :].rearrange("(ks kp) n -> kp ks n", kp=BN))
        b1_sb = const.tile([1, D], f32r)
        nc.sync.dma_start(b1_sb[:], b1_d[:, :])
        b2_sb = const.tile([1, D], f32r)
        nc.sync.dma_start(b2_sb[:], b2_d[:, :])
        bc_sb = const.tile([1, NUM_CLASSES], f32r)
        nc.sync.dma_start(bc_sb[:], bc_d[:, :])
        ones_sb = const.tile([1, BN], f32r)
        nc.sync.dma_start(ones_sb[:], ones_d[:, :])

        # ---- internal DRAM ----
        h1s = dram.tile([SHARD_PAD, D], bf16)  # layer-1 out shard (bf16, AG input)
        hT1 = const.tile([BN, 4, SHARD_PAD], bf16)  # resident transposed h1
        gin = dram.tile([BN, 4 * NUM_GRAPHS], f32)
        gout = dram.tile([BN, 4 * NUM_GRAPHS], f32, addr_space="Shared")

        gacc = const.tile([BN, 4 * NUM_GRAPHS], f32)

        def layer(li, table_bf, self_f32, w1_sb, ws_sb, bias_sb):
            for b in range(NB):
                # gather neighbor rows (bf16)
                g = gpool.tile([BN, T, D], bf16, name="g")
                nc.gpsimd.dma_gather(
                    g[:, :T_lo, :],
                    table_bf[:SPLIT, :],
                    ilo_sb[:, b * T_lo * 8 : (b + 1) * T_lo * 8],
                    BN * T_lo,
                    BN * T_lo,
                    D,
                    single_packet=False,
                    queue_num=(b % 2) if qmode else 0,
                )
                nc.gpsimd.dma_gather(
                    g[:, T_lo:, :],
                    table_bf[SPLIT:, :],
                    ihi_sb[:, b * T_hi * 8 : (b + 1) * T_hi * 8],
                    BN * T_hi,
                    BN * T_hi,
                    D,
                    single_packet=False,
                    queue_num=(b % 2) if qmode else 1,
                )
                # one-hot selection matrices for all T edge tiles
                s_t = spool.tile([BN, T, BN], bf16, name="s_t")
                nc.vector.tensor_tensor(
                    out=s_t[:],
                    in0=slot_sb[:, b * T : (b + 1) * T, None].to_broadcast([BN, T, BN]),
                    in1=iota_bf[:, None, :].to_broadcast([BN, T, BN]),
                    op=mybir.AluOpType.is_equal,
                )
                # segment-sum into PSUM
                ps = ps_nsum.tile([BN, D], f32, name="ps")
                if "edgemm" not in ablate:
                    for t in range(T):
                        nc.tensor.matmul(
                            ps[:],
                            lhsT=s_t[:, t, :],
                            rhs=g[:, t, :],
                            start=(t == 0),
                            stop=(t == T - 1),
                        )
                else:
                    nc.tensor.matmul(
                        ps[:], lhsT=s_t[:, 0, :], rhs=g[:, 0, :], start=True, stop=True
                    )
                # mean
                agg = work.tile([BN, D], f32, name="agg")
                nc.vector.tensor_scalar_mul(agg[:], ps[:], invdeg_sb[:, b : b + 1])
                # transpose agg -> [feat, node] fp32r (4 PE transposes, 1 copy)
                aggT = work.tile([BN, 4, BN], f32r, name="aggT")
                pt = ps_tr.tile([BN, D], f32, name="pt", tag="pt")
                for k in range(4):
                    nc.tensor.transpose(
                        pt[:, k * BN : (k + 1) * BN], agg[:, k * BN : (k + 1) * BN], ident[:]
                    )
                nc.vector.tensor_copy(
                    aggT[:], pt[:].rearrange("p (k n) -> p k n", n=BN)
                )
                if li == 0:
                    # self rows from x (fp32) + transpose
                    hb = work.tile([BN, D], f32, name="hb")
                    nc.sync.dma_start(hb[:], self_f32[b * BN : (b + 1) * BN, :])
                    hbT = work.tile([BN, 4, BN], f32r, name="hbT")
                    pt2 = ps_tr.tile([BN, D], f32, name="pt2", tag="pt")
                    for k in range(4):
                        nc.tensor.transpose(
                            pt2[:, k * BN : (k + 1) * BN],
                            hb[:, k * BN : (k + 1) * BN],
                            ident[:],
                        )
                    nc.vector.tensor_copy(
                        hbT[:], pt2[:].rearrange("p (k n) -> p k n", n=BN)
                    )
                else:
                    hbT = hT1[:, :, b * BN : (b + 1) * BN]
                # dense: out = agg @ W1 + h @ Ws + bias
                po = ps_dense.tile([BN, D], f32, name="po")
                nc.tensor.matmul(
                    po[:], lhsT=ones_sb[:, :], rhs=bias_sb[:, :], start=True, stop=False
                )
                for k in range(4):
                    nc.tensor.matmul(
                        po[:],
                        lhsT=aggT[:, k, :],
                        rhs=w1_sb[:, k, :],
                        start=False,
                        stop=False,
                    )
                for k in range(4):
                    nc.tensor.matmul(
                        po[:],
                        lhsT=hbT[:, k, :],
                        rhs=ws_sb[:, k, :],
                        start=False,
                        stop=(k == 3),
                    )
                # ELU: max(x,0)-1 + exp(min(x,0))
                r = work.tile([BN, D], f32, name="r")
                nc.vector.tensor_scalar(
                    r[:], po[:], 0.0, -1.0, mybir.AluOpType.max, mybir.AluOpType.add
                )
                nmin = work.tile([BN, D], f32, name="nmin")
                nc.vector.tensor_scalar_min(nmin[:], po[:], 0.0)
                e = work.tile([BN, D], f32, name="e")
                nc.scalar.activation(e[:], nmin[:], mybir.ActivationFunctionType.Exp)
                h = hpool.tile([BN, D], f32r, name="h")
                nc.vector.tensor_add(h[:], r[:], e[:])

                if li == 0:
                    h_bf = work.tile([BN, D], bf16, name="h_bf")
                    nc.vector.tensor_copy(h_bf[:], h[:].bitcast(f32))
                    nc.sync.dma_start(h1s[b * BN : (b + 1) * BN, :], h_bf[:])
                    pt3 = ps_tr.tile([BN, D], f32, name="pt3", tag="pt")
                    for k in range(4):
                        nc.tensor.transpose(
                            pt3[:, k * BN : (k + 1) * BN],
                            h[:, k * BN : (k + 1) * BN].bitcast(f32),
                            ident[:],
                        )
                    nc.vector.tensor_copy(
                        hT1[:, :, b * BN : (b + 1) * BN],
                        pt3[:].rearrange("p (k n) -> p k n", n=BN),
                    )
                else:
                    # per-graph pooling: gT += h.T @ onehot(batch)
                    bm = spool.tile([BN, NUM_GRAPHS], f32r, name="bm")
                    nc.vector.tensor_tensor(
                        out=bm[:],
                        in0=bslot_sb[:, b : b + 1].to_broadcast([BN, NUM_GRAPHS]),
                        in1=iota_g[:],
                        op=mybir.AluOpType.is_equal,
                    )
                    pg = ps_pool.tile([BN, 4 * NUM_GRAPHS], f32, name="pg")
                    for k in range(4):
                        nc.tensor.matmul(
                            pg[:, k * NUM_GRAPHS : (k + 1) * NUM_GRAPHS],
                            lhsT=h[:, k * BN : (k + 1) * BN],
                            rhs=bm[:],
                            start=True,
                            stop=True,
                        )
                    if b == 0:
                        nc.vector.tensor_copy(gacc[:], pg[:])
                    else:
                        nc.vector.tensor_add(gacc[:], gacc[:], pg[:])

        for _rep in range(reps):
            h1f = dram.tile(
                [NPAD, D], bf16, addr_space="Shared", name=f"h1f_{_rep}"
            )  # AG output
            layer(0, xg_d, xs_d, w1a_sb, wsa_sb, b1_sb)
            nc.gpsimd.collective_compute(
                "AllGather",
                mybir.AluOpType.bypass,
                replica_groups=[list(range(NC))],
                ins=[h1s[:, :]],
                outs=[h1f[:, :]],
            )
            layer(1, xg_d if "xgonly" in ablate else h1f, None, w1b_sb, wsb_sb, b2_sb)

        # pooled sums all-reduce
        nc.sync.dma_start(gin[:, :], gacc[:])
        nc.gpsimd.collective_compute(
            "AllReduce",
            mybir.AluOpType.add,
            replica_groups=[list(range(NC))],
            ins=[gin[:, :]],
            outs=[gout[:, :]],
        )
        gsum = const.tile([BN, 4, NUM_GRAPHS], f32r)
        gs_raw = const.tile([BN, 4 * NUM_GRAPHS], f32)
        nc.sync.dma_start(gs_raw[:], gout[:, :])
        nc.vector.tensor_tensor(
            out=gsum[:],
            in0=gs_raw[:].rearrange("p (k g) -> p k g", g=NUM_GRAPHS),
            in1=invcnt_sb[:, None, :].to_broadcast([BN, 4, NUM_GRAPHS]),
            op=mybir.AluOpType.mult,
        )
        pf = ps_fin.tile([BN, NUM_CLASSES], f32)
        nc.tensor.matmul(
            pf[:NUM_GRAPHS, :],
            lhsT=ones_sb[:, :NUM_GRAPHS],
            rhs=bc_sb[:, :],
            start=True,
            stop=False,
        )
        for k in range(4):
            nc.tensor.matmul(
                pf[:NUM_GRAPHS, :],
                lhsT=gsum[:, k, :],
                rhs=wc_sb[:, k, :],
                start=False,
                stop=(k == 3),
            )
        o = const.tile([NUM_GRAPHS, NUM_CLASSES], f32)
        nc.vector.tensor_copy(o[:], pf[:NUM_GRAPHS, :])
        nc.sync.dma_start(out_d[:, :], o[:])

    nc.compile()
    return nc


def _make_in_maps(pre):
    w = pre["w"]
    in_maps = []
    for c in range(NC):
        in_maps.append(
            {
                "xg": pre["x_bf"],
                "xs": pre["x_shards"][c],
                "ilo": np.ascontiguousarray(pre["ilo"][c]),
                "ihi": np.ascontiguousarray(pre["ihi"][c]),
                "slot": np.ascontiguousarray(pre["slot"][c]),
                "invdeg": np.ascontiguousarray(pre["invdeg"][c]),
                "bslot": np.ascontiguousarray(pre["bslot"][c]),
                "invcnt": pre["invcnt"],
                "w1a": w["w1a"],
                "wsa": w["wsa"],
                "w1b": w["w1b"],
                "wsb": w["wsb"],
                "wc": w["wc"],
                "b1": w["b1"],
                "b2": w["b2"],
                "bc": w["bc"],
                "ones": w["ones"],
            }
        )
    return in_maps


def _run_spmd(nc, in_maps, repeats=1):
    """Execute on 8 cores via PJRT (axon). Returns (out_core0, exec_times_s)."""
    import jax
    import jax.numpy as jnp  # noqa: F401
    from jax.sharding import Mesh, PartitionSpec, NamedSharding
    from jax.experimental.shard_map import shard_map

    import concourse.mybir as mb
    from concourse.bass2jax import (
        _bass_exec_p,
        install_neuronx_cc_hook,
        partition_id_tensor,
    )

    install_neuronx_cc_hook()
    partition_name = nc.partition_id_tensor.name if nc.partition_id_tensor else None

    in_names, out_names, out_avals, zero_outs = [], [], [], []
    for alloc in nc.m.functions[0].allocations:
        if not isinstance(alloc, mb.MemoryLocationSet):
            continue
        name = alloc.memorylocations[0].name
        if alloc.kind == "ExternalInput":
            if name != partition_name:
                in_names.append(name)
        elif alloc.kind == "ExternalOutput":
            shape = tuple(alloc.tensor_shape)
            dtype = mb.dt.np(alloc.dtype)
            out_names.append(name)
            out_avals.append(jax.core.ShapedArray(shape, dtype))
            zero_outs.append(np.zeros(shape, dtype))
    n_params = len(in_names)
    n_outs = len(out_avals)
    all_in_names = list(in_names) + out_names
    if partition_name is not None:
        all_in_names.append(partition_name)
    donate = tuple(range(n_params, n_params + n_outs))

    def _body(*args):
        operands = list(args)
        if partition_name is not None:
            operands.append(partition_id_tensor())
        outs = _bass_exec_p.bind(
            *operands,
            out_avals=tuple(out_avals),
            in_names=tuple(all_in_names),
            out_names=tuple(out_names),
            lowering_input_output_aliases=(),
            sim_require_finite=True,
            sim_require_nnan=True,
            nc=nc,
        )
        return tuple(outs)

    devices = jax.devices()[:NC]
    mesh = Mesh(np.asarray(devices), ("core",))
    in_specs = (PartitionSpec("core"),) * (n_params + n_outs)
    out_specs = (PartitionSpec("core"),) * len(out_names)
    sharded = jax.jit(
        shard_map(
            _body, mesh=mesh, in_specs=in_specs, out_specs=out_specs, check_rep=False
        ),
        donate_argnums=donate,
        keep_unused=True,
    )
    concat_in = [
        np.concatenate([np.asarray(in_maps[c][nm]) for c in range(NC)], axis=0)
        for nm in in_names
    ]
    shard_spec = NamedSharding(mesh, PartitionSpec("core"))
    concat_in_dev = [jax.device_put(a, shard_spec) for a in concat_in]

    def one_exec():
        zeros = [
            jax.device_put(
                np.zeros((NC * z.shape[0], *z.shape[1:]), z.dtype), shard_spec
            )
            for z in zero_outs
        ]
        t0 = time.perf_counter()
        out_arrs = sharded(*concat_in_dev, *zeros)
        jax.block_until_ready(out_arrs)
        return time.perf_counter() - t0, out_arrs

    times = []
    out_arrs = None
    for _ in range(max(1, repeats)):
        dt_s, out_arrs = one_exec()
        times.append(dt_s)

    outs0 = {
        name: np.asarray(out_arrs[i]).reshape(NC, *out_avals[i].shape)[0]
        for i, name in enumerate(out_names)
    }
    return outs0, times


_CACHE = {}


def _get_compiled(pre, reps=1, ablate=(), qmode=0):
    key = (pre["T"], pre["T_lo"], pre["T_hi"], reps, tuple(ablate), qmode)
    if key not in _CACHE:
        _CACHE[key] = _build(pre["T"], pre["T_lo"], pre["T_hi"], reps, ablate, qmode)
    return _CACHE[key]


def kernel(**inputs) -> np.ndarray:
    pre = _preprocess(**inputs)
    nc = _get_compiled(pre)
    outs, _ = _run_spmd(nc, _make_in_maps(pre), repeats=1)
    return outs["out"].astype(np.float32)


def kernel_timed(inputs, repeats=5, reps=1, ablate=()):
    pre = _preprocess(**inputs)
    nc = _get_compiled(pre, reps, ablate)
    outs, times = _run_spmd(nc, _make_in_maps(pre), repeats=repeats)
    return outs["out"].astype(np.float32), times



# revision 9
# speedup vs baseline: 1.2269x; 1.2269x over previous
"""DEMONetHashGraph Trainium2 kernel — 8-core data-parallel GNN, fp8 edition.

Strategy:
- Fold multi-hash einsum+concat+Wp into one [512,512] weight on host.
- Shard nodes (and their src-sorted outgoing edges) across 8 cores.
- Edge segment-MEAN: host bakes 1/deg into per-edge-tile one-hot matrices
  (fp8), gathers neighbor rows in fp8 (halves DMA), and runs the one-hot
  matmuls in fp8 DoubleRow mode (2 edge tiles per instruction, 0.5 cyc/row).
- Dense (hash+self+bias) also fp8 DoubleRow; activations transposed on PE.
- ELU via the exact identity elu(z) = max(z, min(exp(z),1) - 1).
- Per-graph pooling accumulates in PSUM across all blocks (one matmul/block).
- One fp8 AllGather of h1 between layers; small f32 AllReduce of pools.
"""

import sys

for _p in ("/opt/trn_rl_repo", "/root/.axon_site/_ro/trn_rl_repo"):
    if _p not in sys.path:
        sys.path.insert(0, _p)

import time
from contextlib import ExitStack

import numpy as np

import concourse.bass as bass
import concourse.mybir as mybir
import concourse.tile as tile
from concourse import bacc

# problem constants (hardcoded per spec)
N_NODES = 50000
N_EDGES = 800000
D = 512
NG = 64
NCLS = 10
NC = 8
SHARD = N_NODES // NC  # 6250
BN = 128
NB = (SHARD + BN - 1) // BN  # 49
SHARD_PAD = NB * BN  # 6272
NPAD = NC * SHARD_PAD  # 50176
SPLIT = 32768
GRP = 2  # blocks per gather group

f32 = mybir.dt.float32
f32r = mybir.dt.float32r
bf16 = mybir.dt.bfloat16
i16 = mybir.dt.int16
f8 = mybir.dt.float8e4
F8 = mybir.dt.np(f8)
DR = mybir.MatmulPerfMode.DoubleRow


def _preprocess(x, edge_index, batch, Hm1, Wp1, Ws1, b1, Hm2, Wp2, Ws2, b2, Wc, bc):
    x = np.asarray(x, np.float32)
    src = np.asarray(edge_index[0], np.int64)
    dst = np.asarray(edge_index[1], np.int64)
    batch = np.asarray(batch, np.int64)

    deg = np.bincount(src, minlength=N_NODES)
    iso = np.where(deg == 0)[0]
    if iso.size:
        src = np.concatenate([src, iso])
        dst = np.concatenate([dst, iso])
    invdeg = (1.0 / np.maximum(deg, 1)).astype(np.float32)

    order = np.argsort(src, kind="stable")
    src_s = src[order]
    dst_s = dst[order]
    dst_pad = (dst_s // SHARD) * SHARD_PAD + (dst_s % SHARD)

    blk_starts = [c * SHARD + b * BN for c in range(NC) for b in range(NB)]
    blk_starts.append(N_NODES)
    bounds = np.searchsorted(src_s, np.array(blk_starts))
    lo_mask = dst_pad < SPLIT

    # per-block tile counts: max across cores (SPMD: one program for all 8)
    nlo = np.zeros((NC, NB), np.int64)
    nhi = np.zeros((NC, NB), np.int64)
    for c in range(NC):
        for b in range(NB):
            k = c * NB + b
            m = lo_mask[bounds[k] : bounds[k + 1]]
            nlo[c, b] = int(m.sum())
            nhi[c, b] = int((~m).sum())
    TL = np.maximum(1, -(-nlo.max(axis=0) // BN)).astype(int)  # [NB]
    TH = np.maximum(1, -(-nhi.max(axis=0) // BN)).astype(int)
    TB = TL + TH
    s_off = np.concatenate([[0], np.cumsum(TB)]).astype(int)  # [NB+1]
    TOT = int(s_off[-1])
    LTOT = int(TL.sum())
    HTOT = int(TH.sum())

    groups = [list(range(g, min(g + GRP, NB))) for g in range(0, NB, GRP)]
    lo_off = np.concatenate([[0], np.cumsum(TL)]).astype(int)  # block-prefix of TL
    hi_off = np.concatenate([[0], np.cumsum(TH)]).astype(int)
    TPMAX = max(sum(TB[b] for b in grp) for grp in groups)

    # host-built tables
    s_arr = np.zeros((NC, BN, TOT, BN), np.float32)
    ilo = np.zeros((NC, 16, LTOT * 8), np.int16)
    ihi = np.zeros((NC, 16, HTOT * 8), np.int16)
    for c in range(NC):
        for b in range(NB):
            k = c * NB + b
            e0, e1 = bounds[k], bounds[k + 1]
            m = lo_mask[e0:e1]
            d_blk = dst_pad[e0:e1]
            slot = (src_s[e0:e1] - (c * SHARD + b * BN)).astype(np.int64)
            inv_e = invdeg[src_s[e0:e1]]
            for half, sel in ((0, m), (1, ~m)):
                dd = d_blk[sel] - (SPLIT if half else 0)
                ss = slot[sel]
                vv = inv_e[sel]
                n = len(dd)
                if n == 0:
                    continue
                j = np.arange(n)
                t0 = s_off[b] + (TL[b] if half else 0)
                s_arr[c, j % BN, t0 + j // BN, ss] = 1.0
                arr = ilo if half == 0 else ihi
                colbase = (lo_off[b] if half == 0 else hi_off[b]) * 8
                arr[c, j % 16, colbase + j // 16] = dd.astype(np.int16)
    s_arr = s_arr.astype(F8)
    ilo_r = np.tile(ilo, (1, 8, 1))  # replicate to 128 partitions
    ihi_r = np.tile(ihi, (1, 8, 1))

    # node-indexed per-core tables: xT (feat-major), bm (batch one-hot)
    node = (
        np.arange(NC)[:, None, None] * SHARD
        + np.arange(NB)[None, :, None] * BN
        + np.arange(BN)[None, None, :]
    )  # [NC, NB, BN]
    valid = node < (np.arange(NC)[:, None, None] + 1) * SHARD
    node_c = np.minimum(node, N_NODES - 1)
    # xT[c, p, b, k, n] = x[node(c,b,n), k*128+p]
    xv = np.where(valid[:, :, :, None], x[node_c], 0.0)  # [NC, NB, BN, D]
    xT = (
        xv.reshape(NC, NB, BN, 4, BN)
        .transpose(0, 4, 1, 3, 2)
        .reshape(NC, BN, NB * 4 * BN)
        .astype(F8)
    )
    # invdeg per (core, partition, block) for the per-partition mean scale
    invd = np.where(valid, invdeg[node_c], 1.0).transpose(0, 2, 1).astype(np.float32)
    bslot = np.where(valid, batch[node_c], -1)  # [NC, NB, BN]
    bm = (bslot[:, :, :, None] == np.arange(NG)[None, None, None, :]).astype(F8)
    bm = bm.transpose(0, 2, 1, 3).reshape(NC, BN, NB * NG)

    # padded fp8 gather table for layer-0 x
    x_pad = np.zeros((NPAD, D), np.float32)
    x_pad.reshape(NC, SHARD_PAD, D)[:, :SHARD, :] = x.reshape(NC, SHARD, D)
    x8 = x_pad.astype(F8)

    cnt_g = np.bincount(batch, minlength=NG).astype(np.float32)
    invcnt = (1.0 / np.maximum(cnt_g, 1.0)).reshape(NG, 1).astype(np.float32)

    def fold(Hm, Wp):
        Hcat = np.concatenate([np.asarray(Hm, np.float32)[k] for k in range(4)], axis=1)
        return Hcat @ np.asarray(Wp, np.float32)

    def wpack(W, dt):  # [D, D] -> [128, 4*D] with w[p, k*D+fo] = W[k*128+p, fo]
        W = np.asarray(W, np.float32)
        return W.reshape(4, BN, W.shape[1]).transpose(1, 0, 2).reshape(BN, -1).astype(dt)

    def wsplit(W):  # fp8 main + fp8 residual ~= bf16-grade weights
        W = np.asarray(W, np.float32)
        W8 = W.astype(F8).astype(np.float32)
        return W8.astype(F8), (W - W8).astype(F8)

    W1a8, W1aR = wsplit(fold(Hm1, Wp1))
    Wsa8, WsaR = wsplit(Ws1)
    W1b8, W1bR = wsplit(fold(Hm2, Wp2))
    Wsb8, WsbR = wsplit(Ws2)
    w = dict(
        w1a=wpack(W1a8, F8),
        w1ar=wpack(W1aR, F8),
        wsa=wpack(Wsa8, F8),
        wsar=wpack(WsaR, F8),
        w1b=wpack(W1b8, F8),
        w1br=wpack(W1bR, F8),
        wsb=wpack(Wsb8, F8),
        wsbr=wpack(WsbR, F8),
        wc=wpack(Wc, np.float32),  # [128, 4*10] f32r
        b1=np.asarray(b1, np.float32).reshape(1, D),
        b2=np.asarray(b2, np.float32).reshape(1, D),
        bc=np.asarray(bc, np.float32).reshape(1, NCLS),
        ones=np.ones((1, BN), np.float32),
        ident8=np.eye(BN, dtype=np.float32).astype(F8),
        invcnt=invcnt,
    )
    meta = dict(
        TL=tuple(int(v) for v in TL),
        TH=tuple(int(v) for v in TH),
        TOT=TOT,
        LTOT=LTOT,
        HTOT=HTOT,
        TPMAX=int(TPMAX),
    )
    return dict(
        meta=meta,
        s=s_arr.reshape(NC, BN, TOT * BN),
        invd=invd,
        ilo=ilo_r,
        ihi=ihi_r,
        xT=xT,
        bm=bm,
        x8=x8,
        w=w,
    )


def _build(meta, reps=1, ablate=()):
    TL = np.array(meta["TL"])
    TH = np.array(meta["TH"])
    TB = TL + TH
    s_off = np.concatenate([[0], np.cumsum(TB)]).astype(int)
    lo_off = np.concatenate([[0], np.cumsum(TL)]).astype(int)
    hi_off = np.concatenate([[0], np.cumsum(TH)]).astype(int)
    TOT, LTOT, HTOT, TPMAX = meta["TOT"], meta["LTOT"], meta["HTOT"], meta["TPMAX"]
    groups = [list(range(g, min(g + GRP, NB))) for g in range(0, NB, GRP)]

    nc = bacc.Bacc(
        "TRN2",
        target_bir_lowering=False,
        debug=False,
        num_devices=NC,
        num_swdge_queues=2,
    )
    ein = dict(kind="ExternalInput")
    x8_d = nc.dram_tensor("x8", [NPAD, D], f8, **ein)
    s_d = nc.dram_tensor("s", [BN, TOT * BN], f8, **ein)
    ilo_d = nc.dram_tensor("ilo", [BN, LTOT * 8], i16, **ein)
    ihi_d = nc.dram_tensor("ihi", [BN, HTOT * 8], i16, **ein)
    xT_d = nc.dram_tensor("xT", [BN, NB * 4 * BN], f8, **ein)
    bm_d = nc.dram_tensor("bm", [BN, NB * NG], f8, **ein)
    w1a_d = nc.dram_tensor("w1a", [BN, 4 * D], f8, **ein)
    wsa_d = nc.dram_tensor("wsa", [BN, 4 * D], f8, **ein)
    w1b_d = nc.dram_tensor("w1b", [BN, 4 * D], f8, **ein)
    wsb_d = nc.dram_tensor("wsb", [BN, 4 * D], f8, **ein)
    w1ar_d = nc.dram_tensor("w1ar", [BN, 4 * D], f8, **ein)
    wsar_d = nc.dram_tensor("wsar", [BN, 4 * D], f8, **ein)
    w1br_d = nc.dram_tensor("w1br", [BN, 4 * D], f8, **ein)
    wsbr_d = nc.dram_tensor("wsbr", [BN, 4 * D], f8, **ein)
    wc_d = nc.dram_tensor("wc", [BN, 4 * NCLS], f32r, **ein)
    b1_d = nc.dram_tensor("b1", [1, D], f32r, **ein)
    b2_d = nc.dram_tensor("b2", [1, D], f32r, **ein)
    bc_d = nc.dram_tensor("bc", [1, NCLS], f32r, **ein)
    ones_d = nc.dram_tensor("ones", [1, BN], f32r, **ein)
    id8_d = nc.dram_tensor("id8", [BN, BN], f8, **ein)
    invc_d = nc.dram_tensor("invc", [NG, 1], f32, **ein)
    invd_d = nc.dram_tensor("invd", [BN, NB], f32, **ein)
    out_d = nc.dram_tensor("out", [NG, NCLS], f32, kind="ExternalOutput")

    with tile.TileContext(nc) as tc, ExitStack() as ctx:
        const = ctx.enter_context(tc.tile_pool(name="const", bufs=1))
        dram = ctx.enter_context(tc.tile_pool(name="dram", bufs=1, space="DRAM"))
        gpool = ctx.enter_context(tc.tile_pool(name="gpool", bufs=2))
        xpool = ctx.enter_context(tc.tile_pool(name="xpool", bufs=2))
        work = ctx.enter_context(tc.tile_pool(name="work", bufs=2))
        ps_seg = ctx.enter_context(tc.tile_pool(name="ps_seg", bufs=2, space="PSUM"))
        ps_tr = ctx.enter_context(tc.tile_pool(name="ps_tr", bufs=2, space="PSUM"))
        ps_dense = ctx.enter_context(
            tc.tile_pool(name="ps_dense", bufs=2, space="PSUM")
        )
        ps_pool = ctx.enter_context(tc.tile_pool(name="ps_pool", bufs=1, space="PSUM"))

        # ---- resident constants ----
        s_res = const.tile([BN, TOT, BN], f8)
        nc.sync.dma_start(s_res[:], s_d[:, :].rearrange("p (t n) -> p t n", n=BN))

        def load_w(dram_t, width, dt):
            t = const.tile([BN, 4, width], dt, name=f"w_{dram_t.name}")
            nc.sync.dma_start(
                t[:], dram_t[:, :].rearrange("p (k n) -> p k n", k=4)
            )
            return t

        w1a_sb = load_w(w1a_d, D, f8)
        wsa_sb = load_w(wsa_d, D, f8)
        w1b_sb = load_w(w1b_d, D, f8)
        wsb_sb = load_w(wsb_d, D, f8)
        w1ar_sb = load_w(w1ar_d, D, f8)
        wsar_sb = load_w(wsar_d, D, f8)
        w1br_sb = load_w(w1br_d, D, f8)
        wsbr_sb = load_w(wsbr_d, D, f8)
        wc_sb = load_w(wc_d, NCLS, f32r)
        b1_sb = const.tile([1, D], f32r)
        nc.sync.dma_start(b1_sb[:], b1_d[:, :])
        b2_sb = const.tile([1, D], f32r)
        nc.sync.dma_start(b2_sb[:], b2_d[:, :])
        bc_sb = const.tile([1, NCLS], f32r)
        nc.sync.dma_start(bc_sb[:], bc_d[:, :])
        ones_sb = const.tile([1, BN], f32r)
        nc.sync.dma_start(ones_sb[:], ones_d[:, :])
        id8_sb = const.tile([BN, BN], f8)
        nc.sync.dma_start(id8_sb[:], id8_d[:, :])
        invc_sb = const.tile([NG, 1], f32)
        nc.sync.dma_start(invc_sb[:], invc_d[:, :])
        idb_sb = const.tile([BN, BN], bf16)
        nc.vector.tensor_copy(idb_sb[:], id8_sb[:])
        invd_sb = const.tile([BN, NB], f32)
        nc.sync.dma_start(invd_sb[:], invd_d[:, :])
        hT1 = const.tile([BN, 4, SHARD_PAD], f8)

        h1s = dram.tile([SHARD_PAD, D], f8)
        gin = dram.tile([NG, D], f32)
        gout = dram.tile([NG, D], f32, addr_space="Shared")
        pg = ps_pool.tile([NG, D], f32)

        def seg_matmuls(ps, b, g, g_lo0, g_hi0):
            """Segment-mean matmuls for block b into psum ps."""
            ops = []  # (s_tile_idx, g_tile_idx, n_tiles(1|2))
            for t0s, t0g, tn in (
                (s_off[b], g_lo0, TL[b]),
                (s_off[b] + TL[b], g_hi0, TH[b]),
            ):
                j = 0
                while j + 2 <= tn:
                    ops.append((t0s + j, t0g + j, 2))
                    j += 2
                if j < tn:
                    ops.append((t0s + j, t0g + j, 1))
            if "edgemm" in ablate:
                ops = ops[:1]
            for i, (si, gi, n2) in enumerate(ops):
                if n2 == 2:
                    nc.tensor.matmul(
                        ps[:],
                        lhsT=s_res[:, si : si + 2, :],
                        rhs=g[:, gi : gi + 2, :],
                        start=(i == 0),
                        stop=(i == len(ops) - 1),
                        perf_mode=DR,
                    )
                else:
                    nc.tensor.matmul(
                        ps[:],
                        lhsT=s_res[:, si, :],
                        rhs=g[:, gi, :],
                        start=(i == 0),
                        stop=(i == len(ops) - 1),
                    )

        def layer(li, tbl, w1_pair, ws_pair, bias_sb):
            for grp in groups:
                TLg = int(TL[grp].sum())
                THg = int(TH[grp].sum())
                g = gpool.tile([BN, TPMAX, D], f8, name="g")
                il = xpool.tile([BN, TLg * 8], i16, name="il")
                nc.sync.dma_start(
                    il[:], ilo_d[:, lo_off[grp[0]] * 8 : (lo_off[grp[0]] + TLg) * 8]
                )
                ih = xpool.tile([BN, THg * 8], i16, name="ih")
                nc.sync.dma_start(
                    ih[:], ihi_d[:, hi_off[grp[0]] * 8 : (hi_off[grp[0]] + THg) * 8]
                )
                if "nogather" not in ablate:
                    nc.gpsimd.dma_gather(
                        g[:, :TLg, :],
                        tbl[:SPLIT, :],
                        il[:],
                        BN * TLg,
                        BN * TLg,
                        D,
                        single_packet=False,
                        queue_num=0,
                    )
                    nc.gpsimd.dma_gather(
                        g[:, TLg : TLg + THg, :],
                        tbl[SPLIT:, :],
                        ih[:],
                        BN * THg,
                        BN * THg,
                        D,
                        single_packet=False,
                        queue_num=1,
                    )
                for pos, b in enumerate(grp):
                    g_lo0 = 0 if pos == 0 else int(TL[grp[0]])
                    g_hi0 = TLg + (0 if pos == 0 else int(TH[grp[0]]))
                    ps = ps_seg.tile([BN, D], f32, name="ps")
                    seg_matmuls(ps, b, g, g_lo0, g_hi0)
                    # ps == agg (invdeg baked into s); convert + transpose
                    agg_bf = work.tile([BN, D], bf16, name="agg_bf")
                    nc.vector.tensor_scalar_mul(
                        agg_bf[:], ps[:], invd_sb[:, b : b + 1]
                    )
                    pt = ps_tr.tile([BN, 2 * D], bf16, name="pt", tag="pt")
                    for k in range(4):
                        nc.tensor.transpose(
                            pt[:, k * BN : (k + 1) * BN],
                            agg_bf[:, k * BN : (k + 1) * BN],
                            idb_sb[:],
                        )
                    aggT = work.tile([BN, 4, BN], f8, name="aggT")
                    nc.scalar.activation(
                        aggT[:],
                        pt[:, :D].rearrange("p (k n) -> p k n", n=BN),
                        mybir.ActivationFunctionType.Copy,
                    )
                    if li == 0:
                        sT = xpool.tile([BN, 4, BN], f8, name="xt")
                        nc.sync.dma_start(
                            sT[:],
                            xT_d[:, b * 4 * BN : (b + 1) * 4 * BN].rearrange(
                                "p (k n) -> p k n", n=BN
                            ),
                        )
                    else:
                        sT = hT1[:, :, b * BN : (b + 1) * BN]
                    po = ps_dense.tile([BN, D], f32, name="po")
                    nc.tensor.matmul(
                        po[:],
                        lhsT=ones_sb[:, :],
                        rhs=bias_sb[:, :],
                        start=True,
                        stop="nodense" in ablate,
                    )
                    if "nodense" not in ablate:
                        for w1_sb in w1_pair:
                            for j in range(2):
                                nc.tensor.matmul(
                                    po[:],
                                    lhsT=aggT[:, 2 * j : 2 * j + 2, :],
                                    rhs=w1_sb[:, 2 * j : 2 * j + 2, :],
                                    start=False,
                                    stop=False,
                                    perf_mode=DR,
                                )
                        for wi, ws_sb in enumerate(ws_pair):
                            for j in range(2):
                                nc.tensor.matmul(
                                    po[:],
                                    lhsT=sT[:, 2 * j : 2 * j + 2, :],
                                    rhs=ws_sb[:, 2 * j : 2 * j + 2, :],
                                    start=False,
                                    stop=(wi == len(ws_pair) - 1 and j == 1),
                                    perf_mode=DR,
                                )
                    # ELU(z) = max(z, min(exp(z), 1) - 1)
                    e = work.tile([BN, D], bf16, name="e")
                    nc.scalar.activation(
                        e[:], po[:], mybir.ActivationFunctionType.Exp
                    )
                    tm = work.tile([BN, D], bf16, name="tm")
                    nc.vector.tensor_scalar(
                        tm[:], e[:], 1.0, -1.0, mybir.AluOpType.min, mybir.AluOpType.add
                    )
                    h8 = work.tile([BN, D], f8, name="h8")
                    if li == 0:
                        h_bf = work.tile([BN, D], bf16, name="h_bf")
                        nc.vector.tensor_tensor(
                            out=h_bf[:], in0=po[:], in1=tm[:], op=mybir.AluOpType.max
                        )
                        nc.gpsimd.tensor_copy(h8[:], h_bf[:])
                        nc.sync.dma_start(h1s[b * BN : (b + 1) * BN, :], h8[:])
                        pt3 = ps_tr.tile([BN, 2 * D], bf16, name="pt3", tag="pt")
                        for k in range(4):
                            nc.tensor.transpose(
                                pt3[:, k * BN : (k + 1) * BN],
                                h_bf[:, k * BN : (k + 1) * BN],
                                idb_sb[:],
                            )
                        nc.scalar.activation(
                            hT1[:, :, b * BN : (b + 1) * BN],
                            pt3[:, :D].rearrange("p (k n) -> p k n", n=BN),
                            mybir.ActivationFunctionType.Copy,
                        )
                    else:
                        nc.vector.tensor_tensor(
                            out=h8[:], in0=po[:], in1=tm[:], op=mybir.AluOpType.max
                        )
                        bmt = xpool.tile([BN, NG], f8, name="bmt")
                        nc.sync.dma_start(bmt[:], bm_d[:, b * NG : (b + 1) * NG])
                        nc.tensor.matmul(
                            pg[:],
                            lhsT=bmt[:],
                            rhs=h8[:],
                            start=(b == 0),
                            stop=(b == NB - 1),
                        )

        for _rep in range(reps):
            layer(0, x8_d, (w1a_sb, w1ar_sb), (wsa_sb, wsar_sb), b1_sb)
            if "noag" in ablate:
                layer(1, x8_d, (w1b_sb, w1br_sb), (wsb_sb, wsbr_sb), b2_sb)
            else:
                h1f = dram.tile([NPAD, D], f8, addr_space="Shared", name=f"h1f_{_rep}")
                nc.gpsimd.collective_compute(
                    "AllGather",
                    mybir.AluOpType.bypass,
                    replica_groups=[list(range(NC))],
                    ins=[h1s[:, :]],
                    outs=[h1f[:, :]],
                )
                layer(1, h1f, (w1b_sb, w1br_sb), (wsb_sb, wsbr_sb), b2_sb)

        # ---- tail: pool mean, AllReduce, classifier ----
        pgs = const.tile([NG, D], f32)
        nc.scalar.activation(
            pgs[:], pg[:], mybir.ActivationFunctionType.Copy, scale=invc_sb[:]
        )
        nc.sync.dma_start(gin[:, :], pgs[:])
        nc.gpsimd.collective_compute(
            "AllReduce",
            mybir.AluOpType.add,
            replica_groups=[list(range(NC))],
            ins=[gin[:, :]],
            outs=[gout[:, :]],
        )
        gq = const.tile([NG, D], f32)
        nc.sync.dma_start(gq[:], gout[:, :])
        idr = const.tile([NG, NG], f32)
        nc.vector.tensor_copy(idr[:], id8_sb[:NG, :NG])
        ptf = ps_tr.tile([BN, 2 * NG * 4], f32, name="ptf", tag="pt")
        for k in range(4):
            nc.tensor.transpose(
                ptf[:, k * NG : (k + 1) * NG],
                gq[:, k * BN : (k + 1) * BN],
                idr[:],
            )
        gT = const.tile([BN, 4, NG], f32r)
        nc.vector.tensor_copy(
            gT[:], ptf[:, : 4 * NG].rearrange("p (k n) -> p k n", n=NG)
        )
        pf = ps_tr.tile([NG, NCLS], f32, name="pf", tag="pt")
        nc.tensor.matmul(
            pf[:], lhsT=ones_sb[:, :NG], rhs=bc_sb[:, :], start=True, stop=False
        )
        for k in range(4):
            nc.tensor.matmul(
                pf[:],
                lhsT=gT[:, k, :],
                rhs=wc_sb[:, k, :],
                start=False,
                stop=(k == 3),
            )
        o = const.tile([NG, NCLS], f32)
        nc.vector.tensor_copy(o[:], pf[:])
        nc.sync.dma_start(out_d[:, :], o[:])

    nc.compile()
    return nc


def _make_in_maps(pre):
    w = pre["w"]
    in_maps = []
    for c in range(NC):
        in_maps.append(
            {
                "x8": pre["x8"],
                "s": np.ascontiguousarray(pre["s"][c]),
                "invd": np.ascontiguousarray(pre["invd"][c]),
                "ilo": np.ascontiguousarray(pre["ilo"][c]),
                "ihi": np.ascontiguousarray(pre["ihi"][c]),
                "xT": np.ascontiguousarray(pre["xT"][c]),
                "bm": np.ascontiguousarray(pre["bm"][c]),
                "w1a": w["w1a"],
                "wsa": w["wsa"],
                "w1b": w["w1b"],
                "wsb": w["wsb"],
                "w1ar": w["w1ar"],
                "wsar": w["wsar"],
                "w1br": w["w1br"],
                "wsbr": w["wsbr"],
                "wc": w["wc"],
                "b1": w["b1"],
                "b2": w["b2"],
                "bc": w["bc"],
                "ones": w["ones"],
                "id8": w["ident8"],
                "invc": w["invcnt"],
            }
        )
    return in_maps


def _run_spmd(nc, in_maps, repeats=1):
    """Execute on 8 cores via PJRT (axon). Returns (out_core0, exec_times_s)."""
    import jax
    import jax.numpy as jnp  # noqa: F401
    from jax.sharding import Mesh, PartitionSpec, NamedSharding
    from jax.experimental.shard_map import shard_map

    import concourse.mybir as mb
    from concourse.bass2jax import (
        _bass_exec_p,
        install_neuronx_cc_hook,
        partition_id_tensor,
    )

    install_neuronx_cc_hook()
    partition_name = nc.partition_id_tensor.name if nc.partition_id_tensor else None

    in_names, out_names, out_avals, zero_outs = [], [], [], []
    for alloc in nc.m.functions[0].allocations:
        if not isinstance(alloc, mb.MemoryLocationSet):
            continue
        name = alloc.memorylocations[0].name
        if alloc.kind == "ExternalInput":
            if name != partition_name:
                in_names.append(name)
        elif alloc.kind == "ExternalOutput":
            shape = tuple(alloc.tensor_shape)
            dtype = mb.dt.np(alloc.dtype)
            out_names.append(name)
            out_avals.append(jax.core.ShapedArray(shape, dtype))
            zero_outs.append(np.zeros(shape, dtype))
    n_params = len(in_names)
    n_outs = len(out_avals)
    all_in_names = list(in_names) + out_names
    if partition_name is not None:
        all_in_names.append(partition_name)
    donate = tuple(range(n_params, n_params + n_outs))

    def _body(*args):
        operands = list(args)
        if partition_name is not None:
            operands.append(partition_id_tensor())
        outs = _bass_exec_p.bind(
            *operands,
            out_avals=tuple(out_avals),
            in_names=tuple(all_in_names),
            out_names=tuple(out_names),
            lowering_input_output_aliases=(),
            sim_require_finite=True,
            sim_require_nnan=True,
            nc=nc,
        )
        return tuple(outs)

    devices = jax.devices()[:NC]
    mesh = Mesh(np.asarray(devices), ("core",))
    in_specs = (PartitionSpec("core"),) * (n_params + n_outs)
    out_specs = (PartitionSpec("core"),) * len(out_names)
    sharded = jax.jit(
        shard_map(
            _body, mesh=mesh, in_specs=in_specs, out_specs=out_specs, check_rep=False
        ),
        donate_argnums=donate,
        keep_unused=True,
    )
    concat_in = [
        np.concatenate([np.asarray(in_maps[c][nm]) for c in range(NC)], axis=0)
        for nm in in_names
    ]
    shard_spec = NamedSharding(mesh, PartitionSpec("core"))
    concat_in_dev = [jax.device_put(a, shard_spec) for a in concat_in]

    def one_exec():
        zeros = [
            jax.device_put(
                np.zeros((NC * z.shape[0], *z.shape[1:]), z.dtype), shard_spec
            )
            for z in zero_outs
        ]
        t0 = time.perf_counter()
        out_arrs = sharded(*concat_in_dev, *zeros)
        jax.block_until_ready(out_arrs)
        return time.perf_counter() - t0, out_arrs

    times = []
    out_arrs = None
    for _ in range(max(1, repeats)):
        dt_s, out_arrs = one_exec()
        times.append(dt_s)

    outs0 = {
        name: np.asarray(out_arrs[i]).reshape(NC, *out_avals[i].shape)[0]
        for i, name in enumerate(out_names)
    }
    return outs0, times


_CACHE = {}


def _get_compiled(pre, reps=1, ablate=()):
    key = (tuple(sorted(pre["meta"].items())), reps, tuple(ablate))
    if key not in _CACHE:
        _CACHE[key] = _build(pre["meta"], reps, ablate)
    return _CACHE[key]


def kernel(**inputs) -> np.ndarray:
    pre = _preprocess(**inputs)
    nc = _get_compiled(pre)
    outs, _ = _run_spmd(nc, _make_in_maps(pre), repeats=1)
    return outs["out"].astype(np.float32)


def kernel_timed(inputs, repeats=5, reps=1, ablate=()):
    pre = _preprocess(**inputs)
    nc = _get_compiled(pre, reps, ablate)
    outs, times = _run_spmd(nc, _make_in_maps(pre), repeats=repeats)
    return outs["out"].astype(np.float32), times
